# revision 4
# baseline (speedup 1.0000x reference)
"""DocumentEncoder kernel for Trainium2 (8 NeuronCores, Bass/Tile).

Reference computation (B=256, L=512, D=1024, V=50000):
    emb    = emb_table[tokens + 1]            # [B, L, D] gather
    hidden = emb.mean(axis=1)                 # [B, D]
    scores = einsum('bld,bd->bl', emb, W_b @ hidden)   (algebraic rewrite of
             the ble einsum pair: ~230x less compute)
    attn   = softmax(scores, axis=1)
    ct     = einsum('bl,bld->bd', attn, emb)  # [B, D]

Sharding: data-parallel over B (32 docs/core); W_b replicated; per-core
compact embedding table (only referenced rows, int16-remapped, per-doc
sorted) for the SWDGE dma_gather.

Design (driven by the TimelineSim cost model):
  - table and W_b in bf16: halves the dominant HBM gather traffic
    (16384 rows x 2KB/core); end-to-end rel err ~8e-3 vs 2e-2 budget
  - matmul cost ~ output free size, so every per-doc reduction is phrased
    as free=1-output matmuls: doc sums s^T (lhsT=E chunk, rhs=ones),
    V^T = (W^T s/L)^T (lhsT=wt chunk, rhs=s^T column), scores^T
    (lhsT=E^T chunk, rhs=v^T column), and context u^T (lhsT=E chunk,
    rhs=attn column)
  - no PSUM accumulation chains: each partial gets its own PSUM column in
    one per-doc [128,512] accumulator bank, joined by strided DVE
    tensor_reduce - avoids in-order PE head-of-line stalls on the 175ns
    PSUM write latency
  - E^T comes from PE transposes of bf16 E chunks into single-bank PSUM
    tiles; PSUM->SBUF copies alternate DVE (2x_1p mode) / ACT so both
    engines share the load
  - exp with fused per-partition Z accumulation on ACT; Z broadcast via a
    ones matmul; 1/Z applied on DVE
  - issue order is software-pipelined (front phase of doc b+SKEW before
    the exp-dependent back phase of doc b) so 4-deep engine wait queues
    never block ready work of younger docs
  - u^T scaled tiles batched 4 docs wide, PE-transposed back to row
    layout, and DMAed as contiguous 512B runs
"""

from contextlib import ExitStack

import numpy as np
import ml_dtypes

import concourse.tile as tile
from concourse import bacc, mybir
from concourse.bass_utils import run_bass_kernel_spmd
from concourse.masks import make_identity

B, L, D, V = 256, 512, 1024, 50000
N_CORES = 8
DOCS = B // N_CORES          # 32 docs per core
CB = L // 128                # 4 token blocks per doc
DK = D // 128                # 8 d-chunks

FP32 = mybir.dt.float32
BF16 = mybir.dt.bfloat16
INT16 = mybir.dt.int16
U_MAX = DOCS * L             # compact per-core table rows (16384 < 2**15)
IW = L // 16                 # int16 index columns per doc (32)

# E^T-piece PSUM->SBUF copies alternate DVE/ACT globally (50/50) so both
# engines stay fed regardless of doc scheduling


def build_program():
    nc = bacc.Bacc(
        "TRN2",
        target_bir_lowering=False,
        debug=False,
        num_devices=N_CORES,
    )

    table = nc.dram_tensor("table", [U_MAX, D], BF16, kind="ExternalInput").ap()
    wt = nc.dram_tensor("wt", [D, D], BF16, kind="ExternalInput").ap()
    idx = nc.dram_tensor("idx", [128, DOCS * IW], INT16, kind="ExternalInput").ap()
    out = nc.dram_tensor("out", [DOCS, D], FP32, kind="ExternalOutput").ap()

    with tile.TileContext(nc) as tc, ExitStack() as ctx:
        const = ctx.enter_context(tc.tile_pool(name="const", bufs=1))
        wtp = ctx.enter_context(tc.tile_pool(name="wtp", bufs=1))
        ep = ctx.enter_context(tc.tile_pool(name="ep", bufs=18))
        sb = ctx.enter_context(tc.tile_pool(name="sb", bufs=2))
        etsp = ctx.enter_context(tc.tile_pool(name="etsp", bufs=8))
        etps = ctx.enter_context(tc.tile_pool(name="etps", bufs=4, space="PSUM"))
        accps = ctx.enter_context(tc.tile_pool(name="accps", bufs=4, space="PSUM"))

        # ---- static tiles ----
        idx_sb = const.tile([128, DOCS * IW], INT16, tag="idx")
        nc.sync.dma_start(out=idx_sb[:], in_=idx[:])

        wt_sb = [
            wtp.tile([128, D], BF16, tag=f"wt{k}", name=f"wtt{k}") for k in range(DK)
        ]
        wt_loaded = False

        ident = const.tile([128, 128], FP32, tag="ident")
        make_identity(nc, ident[:])
        identb = const.tile([128, 128], BF16, tag="identb")
        nc.scalar.copy(out=identb[:], in_=ident[:])
        ones_col = const.tile([128, 1], BF16, tag="ones_col")
        nc.vector.memset(ones_col[:], 1.0)
        ones128 = const.tile([128, 128], FP32, tag="ones128")
        nc.vector.memset(ones128[:], 1.0)
        copy_ci = [0]

        def issue_gather(b):
            et = ep.tile([128, CB, D], BF16, tag="e", name=f"et{b}")
            nc.gpsimd.dma_gather(
                out_ap=et[:],
                in_ap=table[:],
                idxs_ap=idx_sb[:, b * IW : (b + 1) * IW],
                num_idxs=L,
                num_idxs_reg=L,
                elem_size=D,
            )
            return et

        def phase1(b, et):
            """sums, E^T transposes+copies, V^T, scores, exp."""
            acc = accps.tile([128, 512], FP32, tag="acc", name=f"acc{b}")

            # flipped sums: 32 unchained partials
            stp = acc[:, 0:32]
            for c in range(CB):
                for k in range(DK):
                    nc.tensor.matmul(
                        out=stp[:, c * DK + k : c * DK + k + 1],
                        lhsT=et[:, c, k * 128 : (k + 1) * 128],
                        rhs=ones_col[:],
                        start=True,
                        stop=True,
                    )
            st_sb = sb.tile([128, DK], BF16, tag="st_sb", bufs=4)
            with nc.allow_low_precision(reason="bf16 s^T feeds bf16 matmul"):
                nc.vector.tensor_reduce(
                    st_sb[:],
                    stp.rearrange("p (c k) -> p k c", k=DK),
                    mybir.AxisListType.X,
                    mybir.AluOpType.add,
                )

            # E^T piece transposes (independent of V^T) + copies to SBUF
            etps_tiles = []
            for kk in range(DK // 2):
                etp = etps.tile([128, 2, CB, 128], BF16, tag="etp")
                for dk in range(2):
                    k = kk * 2 + dk
                    for c in range(CB):
                        nc.tensor.transpose(
                            out=etp[:, dk, c, :],
                            in_=et[:, c, k * 128 : (k + 1) * 128],
                            identity=identb[:],
                        )
                etps_tiles.append(etp)

            ets_tiles = []
            for kk in range(DK // 2):
                ets = etsp.tile([128, 2, CB, 128], BF16, tag="ets", name=f"ets{kk}")
                ci = copy_ci[0] = copy_ci[0] + 1
                eng = "DA"[ci % 2]
                if eng == "A":
                    nc.scalar.copy(out=ets[:], in_=etps_tiles[kk][:])
                elif eng == "P":
                    nc.gpsimd.tensor_copy(out=ets[:], in_=etps_tiles[kk][:])
                else:
                    nc.vector.tensor_copy(out=ets[:], in_=etps_tiles[kk][:])
                ets_tiles.append(ets)

            # per-doc flipped V^T: 64 unchained partials
            vtp = acc[:, 64:128]
            for k in range(DK):
                for m in range(DK):
                    nc.tensor.matmul(
                        out=vtp[:, k * DK + m : k * DK + m + 1],
                        lhsT=wt_sb[k][:, m * 128 : (m + 1) * 128],
                        rhs=st_sb[:, k : k + 1],
                        start=True,
                        stop=True,
                    )
            vt_sb = sb.tile([128, DK], BF16, tag="vt_sb", bufs=4)
            with nc.allow_low_precision(reason="bf16 v^T feeds bf16 matmul"):
                nc.vector.tensor_reduce(
                    vt_sb[:],
                    vtp.rearrange("p (k m) -> p m k", m=DK),
                    mybir.AxisListType.X,
                    mybir.AluOpType.add,
                )

            # scores: 32 unchained partials against v^T columns
            scp = acc[:, 128:160]
            for kk in range(DK // 2):
                for dk in range(2):
                    k = kk * 2 + dk
                    for c in range(CB):
                        nc.tensor.matmul(
                            out=scp[:, c * DK + k : c * DK + k + 1],
                            lhsT=ets_tiles[kk][:, dk, c, :],
                            rhs=vt_sb[:, k : k + 1],
                            start=True,
                            stop=True,
                        )
            sc_sb = sb.tile([128, CB], FP32, tag="sc_sb", bufs=4)
            nc.vector.tensor_reduce(
                sc_sb[:],
                scp.rearrange("p (c k) -> p c k", k=DK),
                mybir.AxisListType.X,
                mybir.AluOpType.add,
            )

            p_sb = sb.tile([128, CB], BF16, tag="p_sb", bufs=4)
            zp = sb.tile([128, 1], FP32, tag="zp", bufs=4)
            nc.scalar.activation(
                out=p_sb[:],
                in_=sc_sb[:],
                func=mybir.ActivationFunctionType.Exp,
                accum_out=zp[:],
            )
            return acc, p_sb, zp

        stg_box = [None]
        pending_fin = []

        def finalize(b, acc, stg):
            tps = acc[0:64, 256:384]
            nc.tensor.transpose(out=tps, in_=stg[:], identity=ident[:])
            og = sb.tile([64, 128], FP32, tag="og", bufs=2)
            nc.vector.tensor_copy(out=og[:], in_=tps)
            nc.sync.dma_start(
                out=out[b - 7 : b + 1, :].rearrange("a (b f) -> (a b) f", f=128),
                in_=og[:],
            )

        def phase2(b, et, acc, p_sb, zp):
            """context, Z, 1/Z scale, quad finalize."""
            utp = acc[:, 160:192]
            for c in range(CB):
                for k in range(DK):
                    nc.tensor.matmul(
                        out=utp[:, c * DK + k : c * DK + k + 1],
                        lhsT=et[:, c, k * 128 : (k + 1) * 128],
                        rhs=p_sb[:, c : c + 1],
                        start=True,
                        stop=True,
                    )
            ut_sb = sb.tile([128, DK], FP32, tag="ut_sb", bufs=4)
            nc.vector.tensor_reduce(
                ut_sb[:],
                utp.rearrange("p (c k) -> p k c", k=DK),
                mybir.AxisListType.X,
                mybir.AluOpType.add,
            )

            z_ps = acc[:, 192:193]
            nc.tensor.matmul(
                out=z_ps, lhsT=ones128[:], rhs=zp[:], start=True, stop=True
            )
            zr = sb.tile([128, 1], FP32, tag="zr", bufs=4)
            nc.vector.reciprocal(out=zr[:], in_=z_ps)

            if b % 8 == 0:
                stg_box[0] = sb.tile([128, 64], FP32, tag="stg", bufs=2, name="stg")
            stg = stg_box[0]
            nc.vector.tensor_scalar(
                out=stg[:, (b % 8) * DK : (b % 8 + 1) * DK],
                in0=ut_sb[:],
                scalar1=zr[:],
                scalar2=None,
                op0=mybir.AluOpType.mult,
            )
            if b % 8 == 7:
                finalize(b, acc, stg)

        # software-pipelined issue order: front phase of doc b+1 lands between
        # doc b's exp and doc b's context, so no engine queue blocks on exp
        SKEW = 4
        ets_all = {}
        state = {}
        for k in range(DK):
            nc.sync.dma_start(out=wt_sb[k][:], in_=wt[k * 128 : (k + 1) * 128, :])
        warm = etps.tile([128, 2, CB, 128], BF16, tag="etp", name="warm")
        for _ in range(56):
            nc.tensor.transpose(out=warm[:, 0, 0, :], in_=identb[:], identity=identb[:])
        for b in range(DOCS):
            ets_all[b] = issue_gather(b)
        for b in range(DOCS + SKEW):
            if b < DOCS:
                state[b] = phase1(b, ets_all[b])
            if b >= SKEW:
                bb = b - SKEW
                acc, p_sb, zp = state.pop(bb)
                phase2(bb, ets_all.pop(bb), acc, p_sb, zp)


    nc.compile()
    return nc


_NC = None


def _get_nc():
    global _NC
    if _NC is None:
        _NC = build_program()
    return _NC


def make_in_maps(tokens, emb_table, W_b):
    tokens = np.asarray(tokens, dtype=np.int64)
    emb_table = np.asarray(emb_table, dtype=np.float32)
    wt_np = np.ascontiguousarray(
        (np.asarray(W_b, dtype=np.float32).T / float(L)).astype(ml_dtypes.bfloat16)
    )

    in_maps = []
    for m in range(N_CORES):
        tok = tokens[m * DOCS : (m + 1) * DOCS]  # [32, 512]
        # compact per-core table: only the rows this core's docs reference,
        # remapped to [0, U) so indices fit the gather ucode's int16 ids
        uniq, inv = np.unique(tok + 1, return_inverse=True)
        assert uniq.size <= U_MAX
        table_np = np.zeros((U_MAX, D), dtype=ml_dtypes.bfloat16)
        table_np[: uniq.size] = emb_table[uniq].astype(ml_dtypes.bfloat16)
        # sort each doc's remapped ids so the gather walks the compact table
        # in ascending order (token order is free under sum/softmax/context)
        inv16 = np.sort(inv.reshape(DOCS, L), axis=1).astype(np.int16)
        # gather ucode reads idx i from partition i%16, column i//16,
        # replicated into each 16-partition group
        blk = inv16.reshape(DOCS, IW, 16).transpose(2, 0, 1)  # [16, DOCS, IW]
        idx_np = np.ascontiguousarray(
            np.tile(blk, (8, 1, 1)).reshape(128, DOCS * IW)
        )
        in_maps.append({"table": table_np, "wt": wt_np, "idx": idx_np})
    return in_maps


def kernel(tokens, max_len, emb_table, W_b):
    assert int(max_len) == L
    nc = _get_nc()
    in_maps = make_in_maps(tokens, emb_table, W_b)
    res = run_bass_kernel_spmd(nc, in_maps, list(range(N_CORES)))
    return np.concatenate([res.results[m]["out"] for m in range(N_CORES)], axis=0)


# revision 5
# speedup vs baseline: 1.0118x; 1.0118x over previous
"""DocumentEncoder kernel for Trainium2 (8 NeuronCores, Bass/Tile).

Reference computation (B=256, L=512, D=1024, V=50000):
    emb    = emb_table[tokens + 1]            # [B, L, D] gather
    hidden = emb.mean(axis=1)                 # [B, D]
    scores = einsum('bld,bd->bl', emb, W_b @ hidden)   (algebraic rewrite of
             the ble einsum pair: ~230x less compute)
    attn   = softmax(scores, axis=1)
    ct     = einsum('bl,bld->bd', attn, emb)  # [B, D]

Sharding: data-parallel over B (32 docs/core); W_b replicated; per-core
compact embedding table (only referenced rows, int16-remapped, per-doc
sorted) for the SWDGE dma_gather.

Design (driven by the TimelineSim cost model):
  - table and W_b in bf16: halves the dominant HBM gather traffic
    (16384 rows x 2KB/core); end-to-end rel err ~8e-3 vs 2e-2 budget
  - matmul cost ~ output free size, so every per-doc reduction is phrased
    as free=1-output matmuls: doc sums s^T (lhsT=E chunk, rhs=ones),
    V^T = (W^T s/L)^T (lhsT=wt chunk, rhs=s^T column), scores^T
    (lhsT=E^T chunk, rhs=v^T column), and context u^T (lhsT=E chunk,
    rhs=attn column)
  - no PSUM accumulation chains: each partial gets its own PSUM column in
    one per-doc [128,512] accumulator bank, joined by strided DVE
    tensor_reduce - avoids in-order PE head-of-line stalls on the 175ns
    PSUM write latency
  - E^T comes from PE transposes of bf16 E chunks into single-bank PSUM
    tiles; PSUM->SBUF copies alternate DVE (2x_1p mode) / ACT so both
    engines share the load
  - exp with fused per-partition Z accumulation on ACT; Z broadcast via a
    ones matmul; 1/Z applied on DVE
  - issue order is software-pipelined (front phase of doc b+SKEW before
    the exp-dependent back phase of doc b) so 4-deep engine wait queues
    never block ready work of younger docs
  - u^T scaled tiles batched 4 docs wide, PE-transposed back to row
    layout, and DMAed as contiguous 512B runs
"""

from contextlib import ExitStack

import numpy as np
import ml_dtypes

import concourse.tile as tile
from concourse import bacc, mybir
from concourse.bass_utils import run_bass_kernel_spmd
from concourse.masks import make_identity

B, L, D, V = 256, 512, 1024, 50000
N_CORES = 8
DOCS = B // N_CORES          # 32 docs per core
CB = L // 128                # 4 token blocks per doc
DK = D // 128                # 8 d-chunks

FP32 = mybir.dt.float32
BF16 = mybir.dt.bfloat16
INT16 = mybir.dt.int16
U_MAX = DOCS * L             # compact per-core table rows (16384 < 2**15)
IW = L // 16                 # int16 index columns per doc (32)

# E^T-piece PSUM->SBUF copies alternate DVE/ACT globally (50/50) so both
# engines stay fed regardless of doc scheduling


def build_program():
    nc = bacc.Bacc(
        "TRN2",
        target_bir_lowering=False,
        debug=False,
        num_devices=N_CORES,
    )

    table = nc.dram_tensor("table", [U_MAX, D], BF16, kind="ExternalInput").ap()
    wt = nc.dram_tensor("wt", [D, D], BF16, kind="ExternalInput").ap()
    idx = nc.dram_tensor("idx", [128, DOCS * IW], INT16, kind="ExternalInput").ap()
    out = nc.dram_tensor("out", [DOCS, D], FP32, kind="ExternalOutput").ap()

    with tile.TileContext(nc) as tc, ExitStack() as ctx:
        const = ctx.enter_context(tc.tile_pool(name="const", bufs=1))
        wtp = ctx.enter_context(tc.tile_pool(name="wtp", bufs=1))
        ep = ctx.enter_context(tc.tile_pool(name="ep", bufs=18))
        sb = ctx.enter_context(tc.tile_pool(name="sb", bufs=2))
        etsp = ctx.enter_context(tc.tile_pool(name="etsp", bufs=8))
        etps = ctx.enter_context(tc.tile_pool(name="etps", bufs=4, space="PSUM"))
        accps = ctx.enter_context(tc.tile_pool(name="accps", bufs=4, space="PSUM"))

        # ---- static tiles ----
        idx_sb = const.tile([128, DOCS * IW], INT16, tag="idx")
        nc.sync.dma_start(out=idx_sb[:], in_=idx[:])

        wt_sb = [
            wtp.tile([128, D], BF16, tag=f"wt{k}", name=f"wtt{k}") for k in range(DK)
        ]
        wt_loaded = False

        ident = const.tile([128, 128], FP32, tag="ident")
        make_identity(nc, ident[:])
        identb = const.tile([128, 128], BF16, tag="identb")
        nc.scalar.copy(out=identb[:], in_=ident[:])
        ones_col = const.tile([128, 1], BF16, tag="ones_col")
        nc.vector.memset(ones_col[:], 1.0)
        ones128 = const.tile([128, 128], FP32, tag="ones128")
        nc.vector.memset(ones128[:], 1.0)
        copy_ci = [0]

        def issue_gather(b):
            et = ep.tile([128, CB, D], BF16, tag="e", name=f"et{b}")
            nc.gpsimd.dma_gather(
                out_ap=et[:],
                in_ap=table[:],
                idxs_ap=idx_sb[:, b * IW : (b + 1) * IW],
                num_idxs=L,
                num_idxs_reg=L,
                elem_size=D,
            )
            return et

        def phase1(b, et):
            """sums, E^T transposes+copies, V^T, scores, exp."""
            acc = accps.tile([128, 512], FP32, tag="acc", name=f"acc{b}")

            # flipped sums: 32 unchained partials
            stp = acc[:, 0:32]
            for c in range(CB):
                for k in range(DK):
                    nc.tensor.matmul(
                        out=stp[:, c * DK + k : c * DK + k + 1],
                        lhsT=et[:, c, k * 128 : (k + 1) * 128],
                        rhs=ones_col[:],
                        start=True,
                        stop=True,
                    )
            st_sb = sb.tile([128, DK], BF16, tag="st_sb", bufs=6)
            with nc.allow_low_precision(reason="bf16 s^T feeds bf16 matmul"):
                nc.vector.tensor_reduce(
                    st_sb[:],
                    stp.rearrange("p (c k) -> p k c", k=DK),
                    mybir.AxisListType.X,
                    mybir.AluOpType.add,
                )

            # E^T piece transposes (independent of V^T) + copies to SBUF
            etps_tiles = []
            for kk in range(DK // 2):
                etp = etps.tile([128, 2, CB, 128], BF16, tag="etp")
                for dk in range(2):
                    k = kk * 2 + dk
                    for c in range(CB):
                        nc.tensor.transpose(
                            out=etp[:, dk, c, :],
                            in_=et[:, c, k * 128 : (k + 1) * 128],
                            identity=identb[:],
                        )
                etps_tiles.append(etp)

            ets_tiles = []
            for kk in range(DK // 2):
                ets = etsp.tile([128, 2, CB, 128], BF16, tag="ets", name=f"ets{kk}")
                ci = copy_ci[0] = copy_ci[0] + 1
                eng = "DA"[ci % 2]
                if eng == "A":
                    nc.scalar.copy(out=ets[:], in_=etps_tiles[kk][:])
                elif eng == "P":
                    nc.gpsimd.tensor_copy(out=ets[:], in_=etps_tiles[kk][:])
                else:
                    nc.vector.tensor_copy(out=ets[:], in_=etps_tiles[kk][:])
                ets_tiles.append(ets)

            # per-doc flipped V^T: 64 unchained partials
            vtp = acc[:, 64:128]
            for k in range(DK):
                for m in range(DK):
                    nc.tensor.matmul(
                        out=vtp[:, k * DK + m : k * DK + m + 1],
                        lhsT=wt_sb[k][:, m * 128 : (m + 1) * 128],
                        rhs=st_sb[:, k : k + 1],
                        start=True,
                        stop=True,
                    )
            vt_sb = sb.tile([128, DK], BF16, tag="vt_sb", bufs=6)
            with nc.allow_low_precision(reason="bf16 v^T feeds bf16 matmul"):
                nc.vector.tensor_reduce(
                    vt_sb[:],
                    vtp.rearrange("p (k m) -> p m k", m=DK),
                    mybir.AxisListType.X,
                    mybir.AluOpType.add,
                )

            # scores: 32 unchained partials against v^T columns
            scp = acc[:, 128:160]
            for kk in range(DK // 2):
                for dk in range(2):
                    k = kk * 2 + dk
                    for c in range(CB):
                        nc.tensor.matmul(
                            out=scp[:, c * DK + k : c * DK + k + 1],
                            lhsT=ets_tiles[kk][:, dk, c, :],
                            rhs=vt_sb[:, k : k + 1],
                            start=True,
                            stop=True,
                        )
            sc_sb = sb.tile([128, CB], FP32, tag="sc_sb", bufs=6)
            nc.vector.tensor_reduce(
                sc_sb[:],
                scp.rearrange("p (c k) -> p c k", k=DK),
                mybir.AxisListType.X,
                mybir.AluOpType.add,
            )

            p_sb = sb.tile([128, CB], BF16, tag="p_sb", bufs=6)
            zp = sb.tile([128, 1], FP32, tag="zp", bufs=6)
            nc.scalar.activation(
                out=p_sb[:],
                in_=sc_sb[:],
                func=mybir.ActivationFunctionType.Exp,
                accum_out=zp[:],
            )
            return acc, p_sb, zp

        stg_box = [None]
        pending_fin = []

        def finalize(b, acc, stg):
            tps = acc[0:64, 256:384]
            nc.tensor.transpose(out=tps, in_=stg[:], identity=ident[:])
            og = sb.tile([64, 128], FP32, tag="og", bufs=2)
            nc.vector.tensor_copy(out=og[:], in_=tps)
            nc.sync.dma_start(
                out=out[b - 7 : b + 1, :].rearrange("a (b f) -> (a b) f", f=128),
                in_=og[:],
            )

        def phase2(b, et, acc, p_sb, zp):
            """context, Z, 1/Z scale, quad finalize."""
            utp = acc[:, 160:192]
            for c in range(CB):
                for k in range(DK):
                    nc.tensor.matmul(
                        out=utp[:, c * DK + k : c * DK + k + 1],
                        lhsT=et[:, c, k * 128 : (k + 1) * 128],
                        rhs=p_sb[:, c : c + 1],
                        start=True,
                        stop=True,
                    )
            ut_sb = sb.tile([128, DK], FP32, tag="ut_sb", bufs=6)
            nc.vector.tensor_reduce(
                ut_sb[:],
                utp.rearrange("p (c k) -> p k c", k=DK),
                mybir.AxisListType.X,
                mybir.AluOpType.add,
            )

            z_ps = acc[:, 192:193]
            nc.tensor.matmul(
                out=z_ps, lhsT=ones128[:], rhs=zp[:], start=True, stop=True
            )
            zr = sb.tile([128, 1], FP32, tag="zr", bufs=6)
            nc.vector.reciprocal(out=zr[:], in_=z_ps)

            if b % 8 == 0:
                stg_box[0] = sb.tile([128, 64], FP32, tag="stg", bufs=2, name="stg")
            stg = stg_box[0]
            nc.vector.tensor_scalar(
                out=stg[:, (b % 8) * DK : (b % 8 + 1) * DK],
                in0=ut_sb[:],
                scalar1=zr[:],
                scalar2=None,
                op0=mybir.AluOpType.mult,
            )
            if b % 8 == 7:
                finalize(b, acc, stg)

        # software-pipelined issue order: front phase of doc b+1 lands between
        # doc b's exp and doc b's context, so no engine queue blocks on exp
        SKEW = 4
        ets_all = {}
        state = {}
        for k in range(DK):
            nc.sync.dma_start(out=wt_sb[k][:], in_=wt[k * 128 : (k + 1) * 128, :])
        warm = etps.tile([128, 2, CB, 128], BF16, tag="etp", name="warm")
        for _ in range(56):
            nc.tensor.transpose(out=warm[:, 0, 0, :], in_=identb[:], identity=identb[:])
        for b in range(DOCS):
            ets_all[b] = issue_gather(b)
        for b in range(DOCS + SKEW):
            if b < DOCS:
                state[b] = phase1(b, ets_all[b])
            if b >= SKEW:
                bb = b - SKEW
                acc, p_sb, zp = state.pop(bb)
                phase2(bb, ets_all.pop(bb), acc, p_sb, zp)


    nc.compile()
    return nc


_NC = None


def _get_nc():
    global _NC
    if _NC is None:
        _NC = build_program()
    return _NC


def make_in_maps(tokens, emb_table, W_b):
    tokens = np.asarray(tokens, dtype=np.int64)
    emb_table = np.asarray(emb_table, dtype=np.float32)
    wt_np = np.ascontiguousarray(
        (np.asarray(W_b, dtype=np.float32).T / float(L)).astype(ml_dtypes.bfloat16)
    )

    in_maps = []
    for m in range(N_CORES):
        tok = tokens[m * DOCS : (m + 1) * DOCS]  # [32, 512]
        # compact per-core table: only the rows this core's docs reference,
        # remapped to [0, U) so indices fit the gather ucode's int16 ids
        uniq, inv = np.unique(tok + 1, return_inverse=True)
        assert uniq.size <= U_MAX
        table_np = np.zeros((U_MAX, D), dtype=ml_dtypes.bfloat16)
        table_np[: uniq.size] = emb_table[uniq].astype(ml_dtypes.bfloat16)
        # sort each doc's remapped ids so the gather walks the compact table
        # in ascending order (token order is free under sum/softmax/context)
        inv16 = np.sort(inv.reshape(DOCS, L), axis=1).astype(np.int16)
        # gather ucode reads idx i from partition i%16, column i//16,
        # replicated into each 16-partition group
        blk = inv16.reshape(DOCS, IW, 16).transpose(2, 0, 1)  # [16, DOCS, IW]
        idx_np = np.ascontiguousarray(
            np.tile(blk, (8, 1, 1)).reshape(128, DOCS * IW)
        )
        in_maps.append({"table": table_np, "wt": wt_np, "idx": idx_np})
    return in_maps


def kernel(tokens, max_len, emb_table, W_b):
    assert int(max_len) == L
    nc = _get_nc()
    in_maps = make_in_maps(tokens, emb_table, W_b)
    res = run_bass_kernel_spmd(nc, in_maps, list(range(N_CORES)))
    return np.concatenate([res.results[m]["out"] for m in range(N_CORES)], axis=0)
